# revision 26
# baseline (speedup 1.0000x reference)
"""Trainium2 Bass kernel for the 3-expert MoE routing MLP (expert-sorted).

Reference computation (B=1M rows):
    y1  = tanh(x @ w1 - b1)                     # [B, 8]
    h_k = sigmoid(y1 @ wa_k - ba_k)             # [B, 16] for experts k=0,1,2
    e_k = h_k @ wb_k - bb_k                     # [B, 32]
    y   = e_{u[b]}  per row b

Strategy: the HOST does the routing.  Rows are sorted by expert id and
packed into a fixed per-core layout: each of the 8 cores gets 63000
columns (2 rows per column: "top" features in partitions 0:64, "bottom"
in 64:128), where columns [e*21000, (e+1)*21000) hold only expert-e rows
(zero-padded; counts are ~41.7k of a 42k quota per core per expert).
The device then runs a dense per-expert MLP with NO masking/gather:

Per quad (4 chunks of 375 columns, single expert by construction):
  * 4x mmA    : [128,16]  x2-chunk -> psA[16k:16k+16]   (2*w1 blocks)
  * 1x ACT    : sigmoid(psA[0:64] - 2*b1) -> T_y4       (tanh via 2sig-1)
  * 1x mmH4   : [64,128] block-diag 2*wa_e over 4 chunks -> psH[0:128]
  * 1x ACT    : sigmoid(psH - ba_e - colsum(wa_e)) -> T_h
  * 2x mmF2   : [64,128] block-diag wb_e over 2 chunks -> psF0/psF1
  * copy+bias : DVE (psF0) and GPSIMD (psF1) tensor_scalar add of -bb_e,
                f32 PSUM -> fp16 SBUF, then one DMA out per quad.

This packs the tiny per-row matmuls across chunks in the partition dim:
1.75 PE cycles/column (vs 3.0 for the mask-based kernel), 2 activation
instructions per 1500 columns (vs 12), and an fp16 output stream.
The host inverts the permutation on the way out.
"""

import math

import numpy as np

import concourse.bass as bass
import concourse.tile as tile
from concourse import mybir
from concourse.bass_utils import run_bass_kernel_spmd

F32 = mybir.dt.float32
F16 = mybir.dt.float16
F8 = mybir.dt.float8e4
NP_F8 = mybir.dt.np(F8)

N_CORES = 8
B = 1_000_000
IN = 64
OUT = 32

CHUNK = 375                  # columns per PSUM tile (375*4B <= 2KB bank)
M_QUADS = 14                 # quads per expert per core
Q_COLS = 4 * CHUNK * M_QUADS     # 21000 columns per expert per core
NQ = 3 * M_QUADS             # 42 quads per core
B_H = 3 * Q_COLS             # 63000 columns per core
CAP = 2 * Q_COLS             # 42000 rows per expert per core


def _pack_weights(w1, b1, w2, b2, w3, b3, w4, b4, w5, b5, w6, b6, w7, b7):
    f32 = np.float32
    was = [w2, w4, w6]
    bas = [b2, b4, b6]
    wbs = [w3, w5, w7]
    bbs = [b3, b5, b7]

    # mmA lhsT [128, 32] fp8e4m3 (x streams in fp8 to halve HBM reads):
    # 2*w1 half-blocks in cols 0:16, zeros in cols 16:32 (PSUM matmul
    # writes must be 32-row aligned, so each chunk's 16 trunk rows are
    # padded to a 32-row group -- PE cost only depends on the moving
    # free size, not output rows).
    wa8 = np.zeros((128, 32), NP_F8)
    wa8[0:64, 0:8] = (2.0 * w1).astype(NP_F8)
    wa8[64:128, 8:16] = (2.0 * w1).astype(NP_F8)

    wpack = np.zeros((128, 32 + 3 * 128 + 3 * 128), np.float16)

    # mmH4 lhsT [128, 128] per expert: 4 chunk blocks of [32 -> 32];
    # block k: rows 32k:32k+8 (y1t top) -> cols 32k:32k+16 (h top),
    # rows 32k+8:32k+16 (y1t bottom) -> cols 32k+16:32k+32; rows
    # 32k+16:32k+32 are the zero-pad rows (zero weights).
    for e in range(3):
        W_h = np.zeros((128, 128), f32)
        for k in range(4):
            r, c = 32 * k, 32 * k
            W_h[r:r + 8, c:c + 16] = 2.0 * was[e]
            W_h[r + 8:r + 16, c + 16:c + 32] = 2.0 * was[e]
        wpack[:, 32 + 128 * e:32 + 128 * (e + 1)] = W_h.astype(np.float16)

    # mmF2 lhsT [64, 128] per expert: 2 chunk blocks of [32 -> 64],
    # each block 2 half-blocks of wb_e [16, 32].  Stored twice (rows
    # 0:64 and 64:128) because PE weights must sit on the same SBUF
    # partitions as the moving operand (th[0:64] / th[64:128]).
    for e in range(3):
        W_f = np.zeros((64, 128), f32)
        for j in range(2):
            r, c = 32 * j, 64 * j
            W_f[r:r + 16, c:c + 32] = wbs[e]
            W_f[r + 16:r + 32, c + 32:c + 64] = wbs[e]
        wf16 = W_f.astype(np.float16)
        wpack[0:64, 416 + 128 * e:416 + 128 * (e + 1)] = wf16
        wpack[64:128, 416 + 128 * e:416 + 128 * (e + 1)] = wf16

    # bpack [128, 7] f32: col 0 = trunk sigmoid bias (-2*b1 per half,
    # tiled over 4 chunk blocks); cols 1..3 = hidden sigmoid bias per
    # expert (-ba_e - colsum(wa_e), the tanh "-1" folded in); cols 4..6 =
    # output bias per expert (-bb_e tiled over chunk/half blocks).
    bpack = np.zeros((128, 7), f32)
    blkA = np.concatenate([-2.0 * b1, -2.0 * b1, np.zeros(16, f32)])  # [32]
    bpack[:, 0] = np.tile(blkA, 4)
    for e in range(3):
        hv = -bas[e] - was[e].sum(axis=0)                    # [16]
        bpack[:, 1 + e] = np.tile(np.concatenate([hv, hv]), 4)
        bpack[:, 4 + e] = np.tile(-bbs[e], 4)
    return dict(wpack=wpack, bpack=bpack, wa8=wa8)


def _split_multi_waits(nc):
    """Walrus codegen allows one sync-wait per instruction; hoist extra
    waits onto same-engine NoOps inserted just before the instruction."""
    n = 0
    for fn in nc.m.functions:
        for blk in fn.blocks:
            out = []
            for ins in blk.instructions:
                si = ins.sync_info
                if si is not None and len(si.on_wait) > 1:
                    waits = list(si.on_wait)
                    for j, w in enumerate(waits[:-1]):
                        nop = mybir.InstNoOp(name=f"{ins.name}-wsplit{j}")
                        nop.engine = ins.engine
                        nop.sync_info = mybir.SyncInfo(on_wait=[w],
                                                       on_update=[])
                        nc.register_instruction(nop)
                        out.append(nop)
                        n += 1
                    si.on_wait = [waits[-1]]
                out.append(ins)
            blk.instructions[:] = out
    return n


def build_nc(chunk=CHUNK, m_quads=M_QUADS):
    nc = bass.Bass("TRN2", target_bir_lowering=False, debug=False)

    nq = 3 * m_quads
    b_h = 4 * chunk * nq
    quad = 4 * chunk

    x8_d = nc.dram_tensor("x8", [128, b_h], F8, kind="ExternalInput").ap()
    wp_d = nc.dram_tensor("wpack", [128, 800], F16, kind="ExternalInput").ap()
    wa8_d = nc.dram_tensor("wa8", [128, 32], F8, kind="ExternalInput").ap()
    bp_d = nc.dram_tensor("bpack", [128, 7], F32, kind="ExternalInput").ap()
    yt_d = nc.dram_tensor("yT", [128, b_h // 2], F16, kind="ExternalOutput").ap()

    SIG = mybir.ActivationFunctionType.Sigmoid
    ADD = mybir.AluOpType.add

    with tile.TileContext(nc) as tc:
        with (
            tc.tile_pool(name="const", bufs=1) as cpool,
            tc.tile_pool(name="xin", bufs=4) as xpool,
            tc.tile_pool(name="ty", bufs=2) as typool,
            tc.tile_pool(name="th", bufs=2) as thpool,
            tc.tile_pool(name="outp", bufs=3) as opool,
            tc.tile_pool(name="psa", bufs=1, space="PSUM") as psapool,
            tc.tile_pool(name="psh", bufs=2, space="PSUM") as pshpool,
            tc.tile_pool(name="psf", bufs=2, space="PSUM") as psfpool,
            tc.tile_pool(name="fill", bufs=1, space="PSUM") as fillpool,
        ):
            wp = cpool.tile([128, 800], F16)
            nc.sync.dma_start(wp[:], wp_d)
            wa8 = cpool.tile([128, 32], F8)
            nc.sync.dma_start(wa8[:], wa8_d)
            bp = cpool.tile([128, 7], F32)
            nc.sync.dma_start(bp[:], bp_d)

            def W_h4(e):
                return wp[:, 32 + 128 * e:32 + 128 * (e + 1)]

            def W_f2(e, j):
                return wp[64 * j:64 * (j + 1),
                          416 + 128 * e:416 + 128 * (e + 1)]

            xt = {}
            ty = {}
            th = {}

            def fetch_x(q):
                if q < nq:
                    xt[q] = xpool.tile([128, quad], F8, name="xt")
                    nc.sync.dma_start(xt[q][:],
                                      x8_d[:, q * quad:(q + 1) * quad])

            fetch_x(0)
            fetch_x(1)

            # Dependency-free PE filler: keeps the HAM activity window
            # busy so the PE clock gate stays at 8/8 (2.4 GHz) instead of
            # the idle-throttled 4/8.  Results are never read.
            def filler(n_cols):
                fl = fillpool.tile([32, chunk], F32, name="fl")
                nc.tensor.matmul(fl[:, 0:n_cols], wp[0:1, 0:32],
                                 wp[0:1, 0:n_cols], start=True, stop=True)

            # HAM warm-up burst: ~4.5us of back-to-back matmuls opens the
            # clock gate (4/8 -> 8/8) before the pipeline starts.
            for _ in range(30):
                filler(chunk)

            # 3-stage software pipeline over quads: per step s the PE runs
            # mmA(s) x4, mmH4(s-1), mmF2(s-2) x2 so its waits are
            # pre-satisfied and the stream stays dense.
            for s in range(nq + 2):
                q0, q1, q2 = s, s - 1, s - 2
                if q0 < nq:
                    fetch_x(q0 + 2)
                    # filler absorbs the psa-bank-reuse wait (bufs=1) and
                    # keeps the PE stream dense for the HAM monitor.
                    filler(chunk)
                    psa = psapool.tile([128, chunk], F32, name="psa")
                    for k in range(4):
                        nc.tensor.matmul(
                            psa[32 * k:32 * (k + 1), :], wa8[:],
                            xt[q0][:, k * chunk:(k + 1) * chunk],
                            start=True, stop=True,
                            tile_position=(0, 32 * k))
                    ty[q0] = typool.tile([128, chunk], F16, name="ty")
                    nc.scalar.activation(ty[q0][:], psa[:], SIG,
                                         bias=bp[0:128, 0:1], scale=1.0)
                    del xt[q0]
                if 0 <= q1 < nq:
                    e1 = q1 // m_quads
                    psh = pshpool.tile([128, chunk], F32, name="psh")
                    nc.tensor.matmul(psh[:], W_h4(e1), ty[q1][:],
                                     start=True, stop=True)
                    th[q1] = thpool.tile([128, chunk], F16, name="th")
                    nc.scalar.activation(th[q1][:], psh[:], SIG,
                                         bias=bp[0:128, 1 + e1:2 + e1],
                                         scale=1.0)
                    del ty[q1]
                if 0 <= q2 < nq:
                    e2 = q2 // m_quads
                    psf0 = psfpool.tile([128, chunk], F32, name="psf0")
                    nc.tensor.matmul(psf0[:], W_f2(e2, 0), th[q2][0:64, :],
                                     start=True, stop=True)
                    psf1 = psfpool.tile([128, chunk], F32, name="psf1")
                    nc.tensor.matmul(psf1[:], W_f2(e2, 1), th[q2][64:128, :],
                                     start=True, stop=True)
                    ot = opool.tile([128, 2 * chunk], F16, name="ot")
                    bb = bp[0:128, 4 + e2:5 + e2]
                    nc.vector.tensor_scalar(ot[:, 0:chunk], psf0[:], bb, None,
                                            ADD)
                    nc.vector.tensor_scalar(ot[:, chunk:2 * chunk], psf1[:],
                                            bb, None, ADD)
                    nc.gpsimd.dma_start(
                        yt_d[:, q2 * 2 * chunk:(q2 + 1) * 2 * chunk], ot[:])
                    del th[q2]

    _split_multi_waits(nc)
    return nc


_NC_CACHE = {}


def _get_nc(chunk=CHUNK, m_quads=M_QUADS):
    key = (chunk, m_quads)
    if key not in _NC_CACHE:
        _NC_CACHE[key] = build_nc(*key)
    return _NC_CACHE[key]


_PACK_STATE = {}


def make_in_maps(x, u, weights, n_cores=N_CORES):
    """Sort rows by expert, pack into per-core fixed-quota layouts."""
    packed = _pack_weights(*weights)
    b = x.shape[0]

    order = np.argsort(u, kind="stable")
    counts = np.bincount(u, minlength=3)
    if counts.max() > n_cores * CAP:
        _PACK_STATE["fallback"] = (x, u, weights)
        return None

    # top_idx/bot_idx [n_cores, B_H]: source row per packed slot, -1 = pad.
    top_idx = np.full((n_cores, B_H), -1, np.int64)
    bot_idx = np.full((n_cores, B_H), -1, np.int64)
    start = 0
    for e in range(3):
        idx_e = order[start:start + counts[e]]
        start += counts[e]
        parts = np.array_split(idx_e, n_cores)
        for c in range(n_cores):
            p = parts[c]
            col0 = e * Q_COLS
            n_top = min(len(p), Q_COLS)
            top_idx[c, col0:col0 + n_top] = p[:n_top]
            n_bot = len(p) - n_top
            if n_bot > 0:
                bot_idx[c, col0:col0 + n_bot] = p[n_top:]

    xh = x.astype(NP_F8)
    in_maps = []
    for c in range(n_cores):
        x8 = np.zeros((128, B_H), NP_F8)
        tv = top_idx[c] >= 0
        bv = bot_idx[c] >= 0
        x8[0:64, tv] = xh[top_idx[c][tv]].T
        x8[64:128, bv] = xh[bot_idx[c][bv]].T
        in_maps.append({"x8": x8, **packed})

    _PACK_STATE["fallback"] = None
    _PACK_STATE["top_idx"] = top_idx
    _PACK_STATE["bot_idx"] = bot_idx
    return in_maps


def _numpy_reference(x, u, weights):
    w1, b1, w2, b2, w3, b3, w4, b4, w5, b5, w6, b6, w7, b7 = weights
    y1 = np.tanh(x @ w1 - b1)
    out = np.zeros((x.shape[0], OUT), np.float32)
    for e, (wa, ba, wb, bb) in enumerate(
            [(w2, b2, w3, b3), (w4, b4, w5, b5), (w6, b6, w7, b7)]):
        m = u == e
        h = 1.0 / (1.0 + np.exp(-(y1[m] @ wa - ba)))
        out[m] = h @ wb - bb
    return out


def unpack_outputs(results, n_cores=N_CORES):
    top_idx = _PACK_STATE["top_idx"]
    bot_idx = _PACK_STATE["bot_idx"]
    y = np.empty((B, OUT), np.float32)
    for c in range(n_cores):
        yt = results[c]["yT"]  # [128, B_H/2] fp16
        # rows: (chunk-in-pair, half, feat); cols: (quad, pair, col375)
        arr = yt.reshape(2, 2, 32, NQ, 2, CHUNK)
        # -> (half, quad, pair, cip, col, feat) -> [2, B_H, OUT]
        yp = arr.transpose(1, 3, 4, 0, 5, 2).reshape(2, B_H, OUT)
        tv = top_idx[c] >= 0
        bv = bot_idx[c] >= 0
        y[top_idx[c][tv]] = yp[0][tv]
        y[bot_idx[c][bv]] = yp[1][bv]
    return y


def kernel(x, u, w1, b1, w2, b2, w3, b3, w4, b4, w5, b5, w6, b6, w7, b7):
    x = np.ascontiguousarray(np.asarray(x, np.float32))
    u = np.ascontiguousarray(np.asarray(u)).astype(np.int64)
    weights = [np.asarray(t, np.float32) for t in
               (w1, b1, w2, b2, w3, b3, w4, b4, w5, b5, w6, b6, w7, b7)]

    in_maps = make_in_maps(x, u, weights)
    if in_maps is None:  # quota exceeded (shouldn't happen) -> host math
        return _numpy_reference(x, u, weights)
    nc = _get_nc()
    res = run_bass_kernel_spmd(nc, in_maps, core_ids=list(range(N_CORES)))
    return unpack_outputs(res.results)


# revision 31
# speedup vs baseline: 1.1527x; 1.1527x over previous
"""Trainium2 Bass kernel for the 3-expert MoE routing MLP (expert-sorted).

Reference computation (B=1M rows):
    y1  = tanh(x @ w1 - b1)                     # [B, 8]
    h_k = sigmoid(y1 @ wa_k - ba_k)             # [B, 16] for experts k=0,1,2
    e_k = h_k @ wb_k - bb_k                     # [B, 32]
    y   = e_{u[b]}  per row b

Strategy: the HOST does the routing.  Rows are sorted by expert id and
packed into a fixed per-core layout: each of the 8 cores gets 63000
columns (2 rows per column: "top" features in partitions 0:64, "bottom"
in 64:128), where columns [e*21000, (e+1)*21000) hold only expert-e rows
(zero-padded; counts are ~41.7k of a 42k quota per core per expert).
The device then runs a dense per-expert MLP with NO masking/gather:

Per quad (4 chunks of 375 columns, single expert by construction):
  * 4x mmA    : [128,16]  x2-chunk -> psA[16k:16k+16]   (2*w1 blocks)
  * 1x ACT    : sigmoid(psA[0:64] - 2*b1) -> T_y4       (tanh via 2sig-1)
  * 1x mmH4   : [64,128] block-diag 2*wa_e over 4 chunks -> psH[0:128]
  * 1x ACT    : sigmoid(psH - ba_e - colsum(wa_e)) -> T_h
  * 2x mmF2   : [64,128] block-diag wb_e over 2 chunks -> psF0/psF1
  * copy+bias : DVE (psF0) and GPSIMD (psF1) tensor_scalar add of -bb_e,
                f32 PSUM -> fp16 SBUF, then one DMA out per quad.

This packs the tiny per-row matmuls across chunks in the partition dim:
1.75 PE cycles/column (vs 3.0 for the mask-based kernel), 2 activation
instructions per 1500 columns (vs 12), and an fp16 output stream.
The host inverts the permutation on the way out.
"""

import math

import numpy as np

import concourse.bass as bass
import concourse.tile as tile
from concourse import mybir
from concourse.bass_utils import run_bass_kernel_spmd

F32 = mybir.dt.float32
F16 = mybir.dt.float16
F8 = mybir.dt.float8e4
NP_F8 = mybir.dt.np(F8)

N_CORES = 8
B = 1_000_000
IN = 64
OUT = 32

CHUNK = 375                  # columns per PSUM tile (375*4B <= 2KB bank)
M_QUADS = 14                 # quads per expert per core
Q_COLS = 4 * CHUNK * M_QUADS     # 21000 columns per expert per core
NQ = 3 * M_QUADS             # 42 quads per core
B_H = 3 * Q_COLS             # 63000 columns per core
CAP = 2 * Q_COLS             # 42000 rows per expert per core


def _pack_weights(w1, b1, w2, b2, w3, b3, w4, b4, w5, b5, w6, b6, w7, b7):
    f32 = np.float32
    was = [w2, w4, w6]
    bas = [b2, b4, b6]
    wbs = [w3, w5, w7]
    bbs = [b3, b5, b7]

    # mmA lhsT [128, 32] fp8e4m3 (x streams in fp8 to halve HBM reads):
    # 2*w1 half-blocks in cols 0:16, zeros in cols 16:32 (PSUM matmul
    # writes must be 32-row aligned, so each chunk's 16 trunk rows are
    # padded to a 32-row group -- PE cost only depends on the moving
    # free size, not output rows).
    wa8 = np.zeros((128, 32), NP_F8)
    wa8[0:64, 0:8] = (2.0 * w1).astype(NP_F8)
    wa8[64:128, 8:16] = (2.0 * w1).astype(NP_F8)

    wpack = np.zeros((128, 32 + 3 * 128 + 3 * 128), np.float16)

    # mmH4 lhsT [128, 128] per expert: 4 chunk blocks of [32 -> 32];
    # block k: rows 32k:32k+8 (y1t top) -> cols 32k:32k+16 (h top),
    # rows 32k+8:32k+16 (y1t bottom) -> cols 32k+16:32k+32; rows
    # 32k+16:32k+32 are the zero-pad rows (zero weights).
    for e in range(3):
        W_h = np.zeros((128, 128), f32)
        for k in range(4):
            r, c = 32 * k, 32 * k
            W_h[r:r + 8, c:c + 16] = 2.0 * was[e]
            W_h[r + 8:r + 16, c + 16:c + 32] = 2.0 * was[e]
        wpack[:, 32 + 128 * e:32 + 128 * (e + 1)] = W_h.astype(np.float16)

    # mmF2 lhsT [64, 128] per expert: 2 chunk blocks of [32 -> 64],
    # each block 2 half-blocks of wb_e [16, 32].  Stored twice (rows
    # 0:64 and 64:128) because PE weights must sit on the same SBUF
    # partitions as the moving operand (th[0:64] / th[64:128]).
    for e in range(3):
        W_f = np.zeros((64, 128), f32)
        for j in range(2):
            r, c = 32 * j, 64 * j
            W_f[r:r + 16, c:c + 32] = wbs[e]
            W_f[r + 16:r + 32, c + 32:c + 64] = wbs[e]
        wf16 = W_f.astype(np.float16)
        wpack[0:64, 416 + 128 * e:416 + 128 * (e + 1)] = wf16
        wpack[64:128, 416 + 128 * e:416 + 128 * (e + 1)] = wf16

    # bpack [128, 7] f32: col 0 = trunk sigmoid bias (-2*b1 per half,
    # tiled over 4 chunk blocks); cols 1..3 = hidden sigmoid bias per
    # expert (-ba_e - colsum(wa_e), the tanh "-1" folded in); cols 4..6 =
    # output bias per expert (-bb_e tiled over chunk/half blocks).
    bpack = np.zeros((128, 7), f32)
    blkA = np.concatenate([-2.0 * b1, -2.0 * b1, np.zeros(16, f32)])  # [32]
    bpack[:, 0] = np.tile(blkA, 4)
    for e in range(3):
        hv = -bas[e] - was[e].sum(axis=0)                    # [16]
        bpack[:, 1 + e] = np.tile(np.concatenate([hv, hv]), 4)
        bpack[:, 4 + e] = np.tile(-bbs[e], 4)
    return dict(wpack=wpack, bpack=bpack, wa8=wa8)


def _split_multi_waits(nc):
    """Walrus codegen allows one sync-wait per instruction; hoist extra
    waits onto same-engine NoOps inserted just before the instruction."""
    n = 0
    for fn in nc.m.functions:
        for blk in fn.blocks:
            out = []
            for ins in blk.instructions:
                si = ins.sync_info
                if si is not None and len(si.on_wait) > 1:
                    waits = list(si.on_wait)
                    for j, w in enumerate(waits[:-1]):
                        nop = mybir.InstNoOp(name=f"{ins.name}-wsplit{j}")
                        nop.engine = ins.engine
                        nop.sync_info = mybir.SyncInfo(on_wait=[w],
                                                       on_update=[])
                        nc.register_instruction(nop)
                        out.append(nop)
                        n += 1
                    si.on_wait = [waits[-1]]
                out.append(ins)
            blk.instructions[:] = out
    return n


def build_nc(chunk=CHUNK, m_quads=M_QUADS):
    nc = bass.Bass("TRN2", target_bir_lowering=False, debug=False)

    nq = 3 * m_quads
    b_h = 4 * chunk * nq
    quad = 4 * chunk

    x8_d = nc.dram_tensor("x8", [128, b_h], F8, kind="ExternalInput").ap()
    wp_d = nc.dram_tensor("wpack", [128, 800], F16, kind="ExternalInput").ap()
    wa8_d = nc.dram_tensor("wa8", [128, 32], F8, kind="ExternalInput").ap()
    bp_d = nc.dram_tensor("bpack", [128, 7], F32, kind="ExternalInput").ap()
    yt_d = nc.dram_tensor("yT", [128, b_h // 2], F16, kind="ExternalOutput").ap()

    SIG = mybir.ActivationFunctionType.Sigmoid
    ADD = mybir.AluOpType.add

    with tile.TileContext(nc) as tc:
        with (
            tc.tile_pool(name="const", bufs=1) as cpool,
            tc.tile_pool(name="xin", bufs=4) as xpool,
            tc.tile_pool(name="ty", bufs=2) as typool,
            tc.tile_pool(name="th", bufs=2) as thpool,
            tc.tile_pool(name="outp", bufs=3) as opool,
            tc.tile_pool(name="psa", bufs=2, space="PSUM") as psapool,
            tc.tile_pool(name="psh", bufs=2, space="PSUM") as pshpool,
            tc.tile_pool(name="psf", bufs=2, space="PSUM") as psfpool,
        ):
            # Issue constant DMAs on gpsimd so they run in parallel with
            # the SP-issued x prefetches (each SP dma_start costs ~565ns
            # of sequencer time on the critical startup path).
            wa8 = cpool.tile([128, 32], F8)
            bp = cpool.tile([128, 7], F32)
            wp = cpool.tile([128, 800], F16)

            def W_h4(e):
                return wp[:, 32 + 128 * e:32 + 128 * (e + 1)]

            def W_f2(e, j):
                return wp[64 * j:64 * (j + 1),
                          416 + 128 * e:416 + 128 * (e + 1)]

            xt = {}
            ty = {}
            th = {}

            def fetch_x(q):
                if q < nq:
                    xt[q] = xpool.tile([128, quad], F8, name="xt")
                    nc.sync.dma_start(xt[q][:],
                                      x8_d[:, q * quad:(q + 1) * quad])

            fetch_x(0)
            nc.gpsimd.dma_start(wa8[:], wa8_d)
            nc.gpsimd.dma_start(bp[:], bp_d)
            nc.gpsimd.dma_start(wp[:], wp_d)
            fetch_x(1)

            # 3-stage software pipeline over quads: per step s the PE runs
            # mmA(s) x4, mmH4(s-1), mmF2(s-2) x2 so its waits are
            # pre-satisfied and the stream stays dense.
            for s in range(nq + 2):
                q0, q1, q2 = s, s - 1, s - 2
                if q0 < nq:
                    fetch_x(q0 + 2)
                    psa = psapool.tile([128, chunk], F32, name="psa")
                    for k in range(4):
                        nc.tensor.matmul(
                            psa[32 * k:32 * (k + 1), :], wa8[:],
                            xt[q0][:, k * chunk:(k + 1) * chunk],
                            start=True, stop=True,
                            tile_position=(0, 32 * k))
                    ty[q0] = typool.tile([128, chunk], F16, name="ty")
                    nc.scalar.activation(ty[q0][:], psa[:], SIG,
                                         bias=bp[0:128, 0:1], scale=1.0)
                    del xt[q0]
                if 0 <= q1 < nq:
                    e1 = q1 // m_quads
                    psh = pshpool.tile([128, chunk], F32, name="psh")
                    nc.tensor.matmul(psh[:], W_h4(e1), ty[q1][:],
                                     start=True, stop=True)
                    th[q1] = thpool.tile([128, chunk], F16, name="th")
                    nc.scalar.activation(th[q1][:], psh[:], SIG,
                                         bias=bp[0:128, 1 + e1:2 + e1],
                                         scale=1.0)
                    del ty[q1]
                if 0 <= q2 < nq:
                    e2 = q2 // m_quads
                    psf0 = psfpool.tile([128, chunk], F32, name="psf0")
                    nc.tensor.matmul(psf0[:], W_f2(e2, 0), th[q2][0:64, :],
                                     start=True, stop=True)
                    psf1 = psfpool.tile([128, chunk], F32, name="psf1")
                    nc.tensor.matmul(psf1[:], W_f2(e2, 1), th[q2][64:128, :],
                                     start=True, stop=True)
                    ot = opool.tile([128, 2 * chunk], F16, name="ot")
                    bb = bp[0:128, 4 + e2:5 + e2]
                    nc.vector.tensor_scalar(ot[:, 0:chunk], psf0[:], bb, None,
                                            ADD)
                    nc.gpsimd.dma_start(
                        yt_d[:, q2 * 2 * chunk:q2 * 2 * chunk + chunk],
                        ot[:, 0:chunk])
                    nc.vector.tensor_scalar(ot[:, chunk:2 * chunk], psf1[:],
                                            bb, None, ADD)
                    nc.sync.dma_start(
                        yt_d[:, q2 * 2 * chunk + chunk:(q2 + 1) * 2 * chunk],
                        ot[:, chunk:2 * chunk])
                    del th[q2]

    _split_multi_waits(nc)
    return nc


_NC_CACHE = {}


def _get_nc(chunk=CHUNK, m_quads=M_QUADS):
    key = (chunk, m_quads)
    if key not in _NC_CACHE:
        _NC_CACHE[key] = build_nc(*key)
    return _NC_CACHE[key]


_PACK_STATE = {}


def make_in_maps(x, u, weights, n_cores=N_CORES):
    """Sort rows by expert, pack into per-core fixed-quota layouts."""
    packed = _pack_weights(*weights)
    b = x.shape[0]

    order = np.argsort(u, kind="stable")
    counts = np.bincount(u, minlength=3)
    if counts.max() > n_cores * CAP:
        _PACK_STATE["fallback"] = (x, u, weights)
        return None

    # top_idx/bot_idx [n_cores, B_H]: source row per packed slot, -1 = pad.
    top_idx = np.full((n_cores, B_H), -1, np.int64)
    bot_idx = np.full((n_cores, B_H), -1, np.int64)
    start = 0
    for e in range(3):
        idx_e = order[start:start + counts[e]]
        start += counts[e]
        parts = np.array_split(idx_e, n_cores)
        for c in range(n_cores):
            p = parts[c]
            col0 = e * Q_COLS
            n_top = min(len(p), Q_COLS)
            top_idx[c, col0:col0 + n_top] = p[:n_top]
            n_bot = len(p) - n_top
            if n_bot > 0:
                bot_idx[c, col0:col0 + n_bot] = p[n_top:]

    xh = x.astype(NP_F8)
    in_maps = []
    for c in range(n_cores):
        x8 = np.zeros((128, B_H), NP_F8)
        tv = top_idx[c] >= 0
        bv = bot_idx[c] >= 0
        x8[0:64, tv] = xh[top_idx[c][tv]].T
        x8[64:128, bv] = xh[bot_idx[c][bv]].T
        in_maps.append({"x8": x8, **packed})

    _PACK_STATE["fallback"] = None
    _PACK_STATE["top_idx"] = top_idx
    _PACK_STATE["bot_idx"] = bot_idx
    return in_maps


def _numpy_reference(x, u, weights):
    w1, b1, w2, b2, w3, b3, w4, b4, w5, b5, w6, b6, w7, b7 = weights
    y1 = np.tanh(x @ w1 - b1)
    out = np.zeros((x.shape[0], OUT), np.float32)
    for e, (wa, ba, wb, bb) in enumerate(
            [(w2, b2, w3, b3), (w4, b4, w5, b5), (w6, b6, w7, b7)]):
        m = u == e
        h = 1.0 / (1.0 + np.exp(-(y1[m] @ wa - ba)))
        out[m] = h @ wb - bb
    return out


def unpack_outputs(results, n_cores=N_CORES):
    top_idx = _PACK_STATE["top_idx"]
    bot_idx = _PACK_STATE["bot_idx"]
    y = np.empty((B, OUT), np.float32)
    for c in range(n_cores):
        yt = results[c]["yT"]  # [128, B_H/2] fp16
        # rows: (chunk-in-pair, half, feat); cols: (quad, pair, col375)
        arr = yt.reshape(2, 2, 32, NQ, 2, CHUNK)
        # -> (half, quad, pair, cip, col, feat) -> [2, B_H, OUT]
        yp = arr.transpose(1, 3, 4, 0, 5, 2).reshape(2, B_H, OUT)
        tv = top_idx[c] >= 0
        bv = bot_idx[c] >= 0
        y[top_idx[c][tv]] = yp[0][tv]
        y[bot_idx[c][bv]] = yp[1][bv]
    return y


def kernel(x, u, w1, b1, w2, b2, w3, b3, w4, b4, w5, b5, w6, b6, w7, b7):
    x = np.ascontiguousarray(np.asarray(x, np.float32))
    u = np.ascontiguousarray(np.asarray(u)).astype(np.int64)
    weights = [np.asarray(t, np.float32) for t in
               (w1, b1, w2, b2, w3, b3, w4, b4, w5, b5, w6, b6, w7, b7)]

    in_maps = make_in_maps(x, u, weights)
    if in_maps is None:  # quota exceeded (shouldn't happen) -> host math
        return _numpy_reference(x, u, weights)
    nc = _get_nc()
    res = run_bass_kernel_spmd(nc, in_maps, core_ids=list(range(N_CORES)))
    return unpack_outputs(res.results)
